# revision 41
# baseline (speedup 1.0000x reference)
"""Causal GQA attention on 8 Trainium2 NeuronCores.

Problem: q [2048, 32, 128], k/v [2048, 8, 128] fp32; out = causal softmax
attention with GQA (4 query heads per KV head), scale 1/sqrt(128).

Sharding: tensor-parallel on the head axis — core c gets query heads
4c..4c+3 and KV head c (GQA groups stay co-located). No collectives.

Per-core kernel (host preps layout: Q^T/K^T transposed + bf16 cast, part of
the sharding step):
  - scores computed TRANSPOSED, ST[k, (head, q)] = K^T-tile contracted with
    Q^T over d: one 128x512 bf16 matmul per (qblock, ktile).
  - exp is split across two engines to keep it off the critical path:
    ~2/3 of k-tile pairs on the scalar engine (exact exp activation, scale
    folded in), ~1/3 on the vector engine via a Schraudolph-style bit trick:
    bf16(exp(x)) ~= bitcast_bf16(int16(round(x*SCALE*128*log2e + B))) —
    one fused multiply-add tensor_scalar op; B centers the multiplicative
    error (range ±2.2%, geometric mean ~1) so mixing with exact-exp tiles
    stays unbiased. End-to-end rel err ~5e-3 (vs 2.9e-3 all-exact).
  - causal diagonal masked by zeroing exp entries via gpsimd affine_select.
  - PV: out[q, 0:129] += expST[:, h]^T @ [V | 1] accumulated in PSUM
    over k-tiles; column 128 accumulates the softmax denominator for free.
  - normalize on the vector engine (reciprocal + broadcast multiply),
    stored as bf16 (host upcasts; halves the output DMA).
Query blocks run in descending causal-length order so the PE sees a dense
matmul stream immediately (HAM warms early). Emission is software-pipelined
(QK of unit u, exp of u-1, PV of u-2) so the in-order PE queue never
head-of-line-blocks on a fresh exp; startup loads are spread across all
three DMA rings ordered by first-use time.
"""
from contextlib import ExitStack

import numpy as np
import ml_dtypes

import concourse.tile as tile
import concourse.mybir as mybir
from concourse import bacc
from concourse.bass_utils import run_bass_kernel_spmd

F32 = mybir.dt.float32
BF16 = mybir.dt.bfloat16
I16 = mybir.dt.int16

S = 2048
H = 32          # total query heads
KVH = 8         # total KV heads
HQ = 4          # query heads per core
D = 128
NT = S // 128   # 16 query/key tiles
NCORES = 8
SCALE = 0.08838834764831845
GRP = 2         # k-tiles per exp/score unit

# Schraudolph constants: y = score*A_S + B_S as int16, bitcast bf16.
A_S = SCALE * 128.0 * 1.4426950408889634   # 16.32224...
B_S = 127.0 * 128.0 - 7.4                  # mean-centering offset

DVE_PAT = (1, 3, 6)  # units (mod 8) whose exp runs on the vector engine
NWARM = 5            # PE warm-up dummy matmuls (fill the initial DMA wait)


def _build_nc():
    nc = bacc.Bacc("TRN2", target_bir_lowering=False)
    qtd = nc.dram_tensor("qt", [128, HQ, NT, 128], BF16, kind="ExternalInput")
    ktd = nc.dram_tensor("kt", [128, NT, 128], BF16, kind="ExternalInput")
    vd = nc.dram_tensor("v", [S, D], BF16, kind="ExternalInput")
    out = nc.dram_tensor("out", [S, HQ, D], BF16, kind="ExternalOutput")

    v3 = vd.ap().rearrange("(t p) d -> p t d", p=128)
    qblocks = list(range(NT - 1, -1, -1))  # biggest causal span first

    with tile.TileContext(nc) as tc, ExitStack() as ctx:
        big = ctx.enter_context(tc.tile_pool(name="big", bufs=1))
        qt = big.tile([128, HQ, NT, 128], BF16)  # [d, h, g(=NT-1-qb), q]
        kt = big.tile([128, NT, 128], BF16)      # [d, kblk, k]
        v1 = big.tile([128, NT, 132], BF16)      # [k, kblk, 129(+pad)]

        # HAM warm-up: dense dummy matmuls while DMAs stream in.
        dw = big.tile([128, 512], BF16)
        nc.vector.memset(dw[:], 0.0)
        with tc.tile_pool(name="dpool", bufs=1, space="PSUM") as dpool:
            dps = dpool.tile([128, 512], F32)
            for _ in range(NWARM):
                nc.tensor.matmul(dps[:], dw[:, :128], dw[:], start=True,
                                 stop=True)

        # loads spread over three DMA rings (each transfer streams ~45 GB/s),
        # ordered by when the first (16-tile) qblock consumes them. The
        # scalar ring is idle until its first exp (~12us) so it carries the
        # other half of the startup-critical Q/K tiles.
        nc.sync.dma_start(kt[:, 0:2, :], ktd[:, 0:2, :])
        nc.scalar.dma_start(qt[:, :2, 0:1, :], qtd[:, :2, 0:1, :])
        nc.gpsimd.dma_start(qt[:, 2:, 0:1, :], qtd[:, 2:, 0:1, :])
        nc.sync.dma_start(kt[:, 2:4, :], ktd[:, 2:4, :])
        nc.scalar.dma_start(kt[:, 4:6, :], ktd[:, 4:6, :])
        nc.gpsimd.dma_start(v1[:, 0:4, 0:128], v3[:, 0:4, :])
        nc.sync.dma_start(kt[:, 6:8, :], ktd[:, 6:8, :])
        nc.scalar.dma_start(kt[:, 8:10, :], ktd[:, 8:10, :])
        nc.gpsimd.dma_start(v1[:, 4:8, 0:128], v3[:, 4:8, :])
        nc.gpsimd.dma_start(qt[:, :, 1:2, :], qtd[:, :, 1:2, :])
        nc.sync.dma_start(kt[:, 10:12, :], ktd[:, 10:12, :])
        nc.scalar.dma_start(kt[:, 12:14, :], ktd[:, 12:14, :])
        nc.sync.dma_start(kt[:, 14:16, :], ktd[:, 14:16, :])
        nc.gpsimd.dma_start(v1[:, 8:16, 0:128], v3[:, 8:16, :])
        nc.gpsimd.dma_start(qt[:, :, 2:4, :], qtd[:, :, 2:4, :])
        nc.gpsimd.dma_start(qt[:, :, 4:8, :], qtd[:, :, 4:8, :])
        nc.gpsimd.dma_start(qt[:, :, 8:12, :], qtd[:, :, 8:12, :])
        nc.gpsimd.dma_start(qt[:, :, 12:16, :], qtd[:, :, 12:16, :])
        nc.vector.memset(v1[:, :, 128:129], 1.0)

        # warm the ACT exp table (after the scalar ring's DMA issues; any
        # time before the first real exp ~12us in is fine)
        dummy = big.tile([128, 1], F32)
        nc.vector.memset(dummy[:], 0.0)
        dume = big.tile([128, 1], F32)
        nc.scalar.activation(dume[:], dummy[:],
                             mybir.ActivationFunctionType.Exp)

        # Flat unit list across qblocks for a software-pipelined emission:
        # QK of unit u is issued LOOKAHEAD units before its exp/mask/PV, so
        # the in-order PE queue never head-of-line-blocks on a fresh exp.
        units = []
        pv_of_qb = {}
        for qb in qblocks:
            nkt = qb + 1
            for g in range((nkt + GRP - 1) // GRP):
                kts = [i for i in range(GRP * g, GRP * g + GRP) if i < nkt]
                units.append({
                    "qb": qb, "g_q": NT - 1 - qb, "kts": kts,
                    "last": kts[-1] == qb,
                })

        with tc.tile_pool(name="stp", bufs=3, space="PSUM") as stp, \
             tc.tile_pool(name="pvp", bufs=1, space="PSUM") as pvp, \
             tc.tile_pool(name="expp", bufs=6) as expp, \
             tc.tile_pool(name="outp", bufs=4) as outp:

            def emit_qk(u, ucnt):
                qb = u["qb"]
                if qb not in pv_of_qb:
                    pv_of_qb[qb] = [
                        pvp.tile([128, 2, 129], F32, tag=f"pvp{i}",
                                 name=f"pvp{i}_{qb}") for i in range(2)]
                st2 = stp.tile([128, GRP, HQ, 128], F32, tag="st2")
                e2 = expp.tile([128, GRP, HQ, 128], BF16, tag="e2")
                u["st2"], u["e2"] = st2, e2
                for j, kt_i in enumerate(u["kts"]):
                    nc.tensor.matmul(
                        st2[:, j], kt[:, kt_i, :], qt[:, :, u["g_q"], :],
                        start=True, stop=True)

            def emit_exp(u, ucnt):
                n = len(u["kts"])
                st2, e2 = u["st2"], u["e2"]
                if (n == GRP) and (ucnt % 8 in DVE_PAT) and u["qb"] > 3:
                    nc.vector.tensor_scalar(
                        e2[:, 0:n].bitcast(I16), st2[:, 0:n],
                        A_S, B_S, mybir.AluOpType.mult, mybir.AluOpType.add)
                else:
                    nc.scalar.activation(
                        e2[:, 0:n], st2[:, 0:n],
                        mybir.ActivationFunctionType.Exp, scale=SCALE)

            def emit_rest(u, ucnt):
                qb, kts = u["qb"], u["kts"]
                e2 = u["e2"]
                pv = pv_of_qb[qb]
                for j, kt_i in enumerate(kts):
                    if kt_i == qb:
                        # diagonal: zero exp where k_local > q_local
                        nc.gpsimd.affine_select(
                            out=e2[:, j], in_=e2[:, j],
                            compare_op=mybir.AluOpType.is_ge,
                            fill=0.0, base=0,
                            pattern=[[0, HQ], [1, 128]],
                            channel_multiplier=-1)
                    for h in range(HQ):
                        nc.tensor.matmul(
                            pv[h // 2][:, h % 2], e2[:, j, h, :],
                            v1[:, kt_i, 0:129],
                            start=(kt_i == 0 and h % 2 == 0),
                            stop=(kt_i == qb and h % 2 == 1))
                if u["last"]:
                    # normalize + store: out[h] = pv[h] / l[h]; one fully
                    # contiguous 128KB store per qblock (cheap descriptors)
                    ot = outp.tile([128, HQ, 128], BF16, tag="ot")
                    rl = outp.tile([128, HQ, 1], F32, tag="rl")
                    for i in range(2):
                        nc.vector.reciprocal(rl[:, 2 * i:2 * i + 2, 0],
                                             pv[i][:, :, 128])
                        nc.vector.tensor_tensor(
                            ot[:, 2 * i:2 * i + 2, :], pv[i][:, :, 0:128],
                            rl[:, 2 * i:2 * i + 2, :].to_broadcast(
                                (128, 2, 128)),
                            mybir.AluOpType.mult)
                    if qb == 0:
                        # very last store on the (by now idle) scalar ring so
                        # it runs concurrently with qb=1's store on sync
                        nc.scalar.dma_start(
                            out[qb * 128:(qb + 1) * 128, :, :], ot[:])
                    else:
                        nc.sync.dma_start(out[qb * 128:(qb + 1) * 128, :, :],
                                          ot[:])

            for u in range(len(units) + 2):
                if u < len(units):
                    emit_qk(units[u], u)
                if 1 <= u < len(units) + 1:
                    emit_exp(units[u - 1], u - 1)
                if u >= 2:
                    emit_rest(units[u - 2], u - 2)

    nc.finalize()
    return nc


_NC_CACHE = None


def kernel(q, k, v):
    global _NC_CACHE
    q = np.asarray(q, dtype=np.float32)
    k = np.asarray(k, dtype=np.float32)
    v = np.asarray(v, dtype=np.float32)
    assert q.shape == (S, H, D) and k.shape == (S, KVH, D)

    if _NC_CACHE is None:
        _NC_CACHE = _build_nc()
    nc = _NC_CACHE

    in_maps = []
    for c in range(NCORES):
        qs = q[:, c * HQ:(c + 1) * HQ, :].astype(ml_dtypes.bfloat16)
        # Q^T: [d, h, qblk, q] with the qblk axis reversed
        qtn = qs.transpose(2, 1, 0).reshape(D, HQ, NT, 128)[:, :, ::-1, :]
        ktn = k[:, c, :].astype(ml_dtypes.bfloat16).T.reshape(D, NT, 128)
        in_maps.append({
            "qt": np.ascontiguousarray(qtn),
            "kt": np.ascontiguousarray(ktn),
            "v": np.ascontiguousarray(v[:, c, :].astype(ml_dtypes.bfloat16)),
        })

    res = run_bass_kernel_spmd(nc, in_maps, core_ids=list(range(NCORES)))
    full = np.concatenate([res.results[c]["out"] for c in range(NCORES)],
                          axis=1)
    return full.astype(np.float32)


# revision 42
# speedup vs baseline: 1.0180x; 1.0180x over previous
"""Causal GQA attention on 8 Trainium2 NeuronCores.

Problem: q [2048, 32, 128], k/v [2048, 8, 128] fp32; out = causal softmax
attention with GQA (4 query heads per KV head), scale 1/sqrt(128).

Sharding: tensor-parallel on the head axis — core c gets query heads
4c..4c+3 and KV head c (GQA groups stay co-located). No collectives.

Per-core kernel (host preps layout: Q^T/K^T transposed + bf16 cast, part of
the sharding step):
  - scores computed TRANSPOSED, ST[k, (head, q)] = K^T-tile contracted with
    Q^T over d: one 128x512 bf16 matmul per (qblock, ktile).
  - exp is split across two engines to keep it off the critical path:
    ~2/3 of k-tile pairs on the scalar engine (exact exp activation, scale
    folded in), ~1/3 on the vector engine via a Schraudolph-style bit trick:
    bf16(exp(x)) ~= bitcast_bf16(int16(round(x*SCALE*128*log2e + B))) —
    one fused multiply-add tensor_scalar op; B centers the multiplicative
    error (range ±2.2%, geometric mean ~1) so mixing with exact-exp tiles
    stays unbiased. End-to-end rel err ~5e-3 (vs 2.9e-3 all-exact).
  - causal diagonal masked by zeroing exp entries via gpsimd affine_select.
  - PV: out[q, 0:129] += expST[:, h]^T @ [V | 1] accumulated in PSUM
    over k-tiles; column 128 accumulates the softmax denominator for free.
  - normalize on the vector engine (reciprocal + broadcast multiply),
    stored as bf16 (host upcasts; halves the output DMA).
Query blocks run in descending causal-length order so the PE sees a dense
matmul stream immediately (HAM warms early). Emission is software-pipelined
(QK of unit u, exp of u-1, PV of u-2) so the in-order PE queue never
head-of-line-blocks on a fresh exp; startup loads are spread across all
three DMA rings ordered by first-use time.
"""
from contextlib import ExitStack

import numpy as np
import ml_dtypes

import concourse.tile as tile
import concourse.mybir as mybir
from concourse import bacc
from concourse.bass_utils import run_bass_kernel_spmd

F32 = mybir.dt.float32
BF16 = mybir.dt.bfloat16
I16 = mybir.dt.int16

S = 2048
H = 32          # total query heads
KVH = 8         # total KV heads
HQ = 4          # query heads per core
D = 128
NT = S // 128   # 16 query/key tiles
NCORES = 8
SCALE = 0.08838834764831845
GRP = 2         # k-tiles per exp/score unit

# Schraudolph constants: y = score*A_S + B_S as int16, bitcast bf16.
A_S = SCALE * 128.0 * 1.4426950408889634   # 16.32224...
B_S = 127.0 * 128.0 - 7.4                  # mean-centering offset

DVE_PAT = (1, 3, 6)  # units (mod 8) whose exp runs on the vector engine
NWARM = 5            # PE warm-up dummy matmuls (fill the initial DMA wait)


def _build_nc():
    nc = bacc.Bacc("TRN2", target_bir_lowering=False)
    qtd = nc.dram_tensor("qt", [128, HQ, NT, 128], BF16, kind="ExternalInput")
    ktd = nc.dram_tensor("kt", [128, NT, 128], BF16, kind="ExternalInput")
    vd = nc.dram_tensor("v", [S, D], BF16, kind="ExternalInput")
    out = nc.dram_tensor("out", [S, HQ, D], BF16, kind="ExternalOutput")

    v3 = vd.ap().rearrange("(t p) d -> p t d", p=128)
    qblocks = list(range(NT - 1, -1, -1))  # biggest causal span first

    with tile.TileContext(nc) as tc, ExitStack() as ctx:
        big = ctx.enter_context(tc.tile_pool(name="big", bufs=1))
        qt = big.tile([128, HQ, NT, 128], BF16)  # [d, h, g(=NT-1-qb), q]
        kt = big.tile([128, NT, 128], BF16)      # [d, kblk, k]
        v1 = big.tile([128, NT, 132], BF16)      # [k, kblk, 129(+pad)]

        # HAM warm-up: dense dummy matmuls while DMAs stream in.
        dw = big.tile([128, 512], BF16)
        nc.vector.memset(dw[:], 0.0)
        with tc.tile_pool(name="dpool", bufs=1, space="PSUM") as dpool:
            dps = dpool.tile([128, 512], F32)
            for _ in range(NWARM):
                nc.tensor.matmul(dps[:], dw[:, :128], dw[:], start=True,
                                 stop=True)

        # loads spread over three DMA rings (each transfer streams ~45 GB/s),
        # ordered by when the first (16-tile) qblock consumes them. The
        # scalar ring is idle until its first exp (~12us) so it carries the
        # other half of the startup-critical Q/K tiles.
        nc.sync.dma_start(kt[:, 0:2, :], ktd[:, 0:2, :])
        nc.scalar.dma_start(qt[:, :2, 0:1, :], qtd[:, :2, 0:1, :])
        nc.gpsimd.dma_start(qt[:, 2:, 0:1, :], qtd[:, 2:, 0:1, :])
        nc.sync.dma_start(kt[:, 2:4, :], ktd[:, 2:4, :])
        nc.scalar.dma_start(kt[:, 4:6, :], ktd[:, 4:6, :])
        nc.gpsimd.dma_start(v1[:, 0:4, 0:128], v3[:, 0:4, :])
        nc.sync.dma_start(kt[:, 6:8, :], ktd[:, 6:8, :])
        nc.scalar.dma_start(kt[:, 8:10, :], ktd[:, 8:10, :])
        nc.gpsimd.dma_start(v1[:, 4:8, 0:128], v3[:, 4:8, :])
        nc.gpsimd.dma_start(qt[:, :, 1:2, :], qtd[:, :, 1:2, :])
        nc.sync.dma_start(kt[:, 10:12, :], ktd[:, 10:12, :])
        nc.scalar.dma_start(kt[:, 12:14, :], ktd[:, 12:14, :])
        nc.sync.dma_start(kt[:, 14:16, :], ktd[:, 14:16, :])
        nc.gpsimd.dma_start(v1[:, 8:16, 0:128], v3[:, 8:16, :])
        nc.gpsimd.dma_start(qt[:, :, 2:4, :], qtd[:, :, 2:4, :])
        nc.gpsimd.dma_start(qt[:, :, 4:8, :], qtd[:, :, 4:8, :])
        nc.gpsimd.dma_start(qt[:, :, 8:12, :], qtd[:, :, 8:12, :])
        nc.gpsimd.dma_start(qt[:, :, 12:16, :], qtd[:, :, 12:16, :])
        nc.vector.memset(v1[:, :, 128:129], 1.0)

        # warm the ACT exp table (after the scalar ring's DMA issues; any
        # time before the first real exp ~12us in is fine)
        dummy = big.tile([128, 1], F32)
        nc.vector.memset(dummy[:], 0.0)
        dume = big.tile([128, 1], F32)
        nc.scalar.activation(dume[:], dummy[:],
                             mybir.ActivationFunctionType.Exp)

        # Flat unit list across qblocks for a software-pipelined emission:
        # QK of unit u is issued LOOKAHEAD units before its exp/mask/PV, so
        # the in-order PE queue never head-of-line-blocks on a fresh exp.
        units = []
        pv_of_qb = {}
        for qb in qblocks:
            nkt = qb + 1
            for g in range((nkt + GRP - 1) // GRP):
                kts = [i for i in range(GRP * g, GRP * g + GRP) if i < nkt]
                units.append({
                    "qb": qb, "g_q": NT - 1 - qb, "kts": kts,
                    "last": kts[-1] == qb,
                })

        with tc.tile_pool(name="stp", bufs=3, space="PSUM") as stp, \
             tc.tile_pool(name="pvp", bufs=1, space="PSUM") as pvp, \
             tc.tile_pool(name="expp", bufs=6) as expp, \
             tc.tile_pool(name="outp", bufs=4) as outp:

            def emit_qk(u, ucnt):
                qb = u["qb"]
                if qb not in pv_of_qb:
                    pv_of_qb[qb] = [
                        pvp.tile([128, 2, 129], F32, tag=f"pvp{i}",
                                 name=f"pvp{i}_{qb}") for i in range(2)]
                st2 = stp.tile([128, GRP, HQ, 128], F32, tag="st2")
                e2 = expp.tile([128, GRP, HQ, 128], BF16, tag="e2")
                u["st2"], u["e2"] = st2, e2
                for j, kt_i in enumerate(u["kts"]):
                    nc.tensor.matmul(
                        st2[:, j], kt[:, kt_i, :], qt[:, :, u["g_q"], :],
                        start=True, stop=True)

            def emit_exp(u, ucnt):
                n = len(u["kts"])
                st2, e2 = u["st2"], u["e2"]
                if (n == GRP) and (ucnt % 8 in DVE_PAT) and u["qb"] > 3:
                    nc.vector.tensor_scalar(
                        e2[:, 0:n].bitcast(I16), st2[:, 0:n],
                        A_S, B_S, mybir.AluOpType.mult, mybir.AluOpType.add)
                else:
                    nc.scalar.activation(
                        e2[:, 0:n], st2[:, 0:n],
                        mybir.ActivationFunctionType.Exp, scale=SCALE)

            def emit_rest(u, ucnt):
                qb, kts = u["qb"], u["kts"]
                e2 = u["e2"]
                pv = pv_of_qb[qb]
                for j, kt_i in enumerate(kts):
                    if kt_i == qb:
                        # diagonal: zero exp where k_local > q_local
                        nc.gpsimd.affine_select(
                            out=e2[:, j], in_=e2[:, j],
                            compare_op=mybir.AluOpType.is_ge,
                            fill=0.0, base=0,
                            pattern=[[0, HQ], [1, 128]],
                            channel_multiplier=-1)
                    for h in range(HQ):
                        nc.tensor.matmul(
                            pv[h // 2][:, h % 2], e2[:, j, h, :],
                            v1[:, kt_i, 0:129],
                            start=(kt_i == 0 and h % 2 == 0),
                            stop=(kt_i == qb and h % 2 == 1))
                if u["last"]:
                    # normalize + store: out[h] = pv[h] / l[h]; one fully
                    # contiguous 128KB store per qblock (cheap descriptors)
                    ot = outp.tile([128, HQ, 128], BF16, tag="ot")
                    rl = outp.tile([128, HQ, 1], F32, tag="rl")
                    for i in range(2):
                        nc.vector.reciprocal(rl[:, 2 * i:2 * i + 2, 0],
                                             pv[i][:, :, 128])
                        nc.vector.tensor_tensor(
                            ot[:, 2 * i:2 * i + 2, :], pv[i][:, :, 0:128],
                            rl[:, 2 * i:2 * i + 2, :].to_broadcast(
                                (128, 2, 128)),
                            mybir.AluOpType.mult)
                    if qb <= 1:
                        # final two qblocks: split the store across two idle
                        # rings so the drain tail halves
                        nc.sync.dma_start(
                            out[qb * 128:(qb + 1) * 128, 0:2, :],
                            ot[:, 0:2, :])
                        nc.scalar.dma_start(
                            out[qb * 128:(qb + 1) * 128, 2:4, :],
                            ot[:, 2:4, :])
                    else:
                        nc.sync.dma_start(out[qb * 128:(qb + 1) * 128, :, :],
                                          ot[:])

            for u in range(len(units) + 2):
                if u < len(units):
                    emit_qk(units[u], u)
                if 1 <= u < len(units) + 1:
                    emit_exp(units[u - 1], u - 1)
                if u >= 2:
                    emit_rest(units[u - 2], u - 2)

    nc.finalize()
    return nc


_NC_CACHE = None


def kernel(q, k, v):
    global _NC_CACHE
    q = np.asarray(q, dtype=np.float32)
    k = np.asarray(k, dtype=np.float32)
    v = np.asarray(v, dtype=np.float32)
    assert q.shape == (S, H, D) and k.shape == (S, KVH, D)

    if _NC_CACHE is None:
        _NC_CACHE = _build_nc()
    nc = _NC_CACHE

    in_maps = []
    for c in range(NCORES):
        qs = q[:, c * HQ:(c + 1) * HQ, :].astype(ml_dtypes.bfloat16)
        # Q^T: [d, h, qblk, q] with the qblk axis reversed
        qtn = qs.transpose(2, 1, 0).reshape(D, HQ, NT, 128)[:, :, ::-1, :]
        ktn = k[:, c, :].astype(ml_dtypes.bfloat16).T.reshape(D, NT, 128)
        in_maps.append({
            "qt": np.ascontiguousarray(qtn),
            "kt": np.ascontiguousarray(ktn),
            "v": np.ascontiguousarray(v[:, c, :].astype(ml_dtypes.bfloat16)),
        })

    res = run_bass_kernel_spmd(nc, in_maps, core_ids=list(range(NCORES)))
    full = np.concatenate([res.results[c]["out"] for c in range(NCORES)],
                          axis=1)
    return full.astype(np.float32)
